# revision 39
# baseline (speedup 1.0000x reference)
"""Causal self-attention (B=2, T=2048, C=1024, H=16) on 8 TRN2 NeuronCores.

Sharding: core c handles batch b = c//4 and heads 4*(c%4) .. 4*(c%4)+3
(data-parallel over B, tensor-parallel over heads; full K/V for its heads
is computed locally from the core's QKV projection slice).

Per-core dataflow (all matmuls bf16 inputs, fp32 PSUM accumulation):
  - host passes xT = x[b].T [C,T], wqk = [Wq_h|Wk_h]^T [C,512],
    wv = [Wv_h0|...|Wv_h3]^T [C,260] (65-wide blocks, last col zero),
    and a [128,128] causal-mask tile, all bf16, PACKED so each SBUF tile
    [128, n*512] holds all 8 c-contraction blocks side by side -> the
    whole input loads with 8 big DMAs (DGE configs cost ~0.6us each on a
    sequencer; 40 small ones made the old startup issue-bound)
  - qT/kT [128,T] per pair via projection matmuls (contraction c on
    partitions); kT kept UNSPLIT: rows 0:64 = even head, 64:128 = odd
  - S matmuls are K=64 ROW-TILED CONCURRENT PAIRS: the even head's
    matmul runs in PE rows 0:63 (tile_position (0,0)) while the odd
    head's runs in rows 64:127 (tile_position (64,0)) -- the PE runs
    both simultaneously (per-subarray concurrency), so a chunk's two
    S matmuls take ~N cycles total instead of 2N.  Outputs land in the
    two separate PSUM banks of one [128,1024] s tile.
  - s [k=128, 1024] = [S_even|S_odd] per chunk; exp on ScalarE (scale=1/8
    fused) over only the causally-live columns; causal masking of the
    partial diagonal 128-col block is a GpSimd multiply with the
    precomputed mask (ScalarE stays exp-only; DVE keeps the casts)
  - v [t,260] with a ones column appended per head (65th of each block)
  - y^T [65, 512] += V'.T @ P^T accumulated over k-tiles; row 64 = softmax
    denominators (from the ones column)
  - NO on-device normalize: the [65,512] accumulator (y rows + denom row)
    is copied to SBUF and DMA'd out; the host does y[:64]/y[64] in numpy.
    This removes the recip + partition-broadcast + multiply chain that
    used to dominate the kernel tail and the block boundaries.

Schedule: one flat software pipeline over both pairs, block-interleaved
(p0,b0),(p1,b0),(p0,b1),... with PV lagging S by 4 chunks so ScalarE's exp
queue stays full; projection chains are front-loaded into the preceding
sub-block's first chunks.  PSUM: 2 s slots (4 banks) + 2 chain banks +
ye/yo (2 banks) = 8 banks exactly.
"""

import os
import sys
import types
import numpy as np
import ml_dtypes

import concourse.bass as bass
import concourse.mybir as mybir
import concourse.tile as tile
from concourse import bacc
from concourse.bass_utils import run_bass_kernel_spmd

B, T, C, H = 2, 2048, 1024, 16
D = 64
NCORES = 8
HPC = 4          # heads per core
NQB = 4          # q blocks of 512
QB = 512
F32 = mybir.dt.float32
BF16 = mybir.dt.bfloat16
NPBF16 = ml_dtypes.bfloat16
EXP = mybir.ActivationFunctionType.Exp
MULT = mybir.AluOpType.mult


def _install_profhook():
    """Register the NTFF profile hook shim so BASS_TRACE=1 works; harmless
    no-op (graceful trace skip) when the axon .so lacks profiling."""
    if "antenv.axon_hooks" not in sys.modules:
        mod = types.ModuleType("antenv.axon_hooks")
        mod._hook = None
        mod.set_axon_ntff_profile_hook = lambda h: setattr(mod, "_hook", h)
        mod.get_axon_ntff_profile_hook = lambda: mod._hook
        sys.modules["antenv.axon_hooks"] = mod
        try:
            import antenv
            antenv.axon_hooks = mod
        except ImportError:
            pass
    try:
        from trn_agent_boot.trn_boot import _ntff_profile_via_ctypes
        sys.modules["antenv.axon_hooks"].set_axon_ntff_profile_hook(
            _ntff_profile_via_ctypes("/opt/axon/libaxon_pjrt.so")
        )
        import concourse.bass_utils as bu
        bu.upload_artifacts = lambda tmpdir: tmpdir
    except Exception:
        pass


_install_profhook()

_NC = None


def _build():
    nc = bacc.Bacc("TRN2", target_bir_lowering=False, debug=False,
                   num_devices=NCORES)
    xT_d = nc.declare_dram_parameter("xT", [4, 128, 4096], BF16,
                                     isOutput=False)
    wqk_d = nc.declare_dram_parameter("wqk", [128, 4096], BF16,
                                      isOutput=False)
    wv_d = nc.declare_dram_parameter("wv", [128, 2080], BF16,
                                     isOutput=False)
    cm_d = nc.declare_dram_parameter("cm", [128, 128], BF16, isOutput=False)
    y_d = nc.declare_dram_parameter("y", [HPC, 65, T], BF16, isOutput=True)
    dbg_d = nc.declare_dram_parameter("dbg", [1, 12], F32, isOutput=True)

    from contextlib import ExitStack
    with tile.TileContext(nc) as tc, ExitStack() as ctx:
        sb = ctx.enter_context(tc.tile_pool(name="sb", bufs=1))
        pp = ctx.enter_context(tc.tile_pool(name="pp", bufs=10))
        yp = ctx.enter_context(tc.tile_pool(name="yp", bufs=4))
        # dedicated pools: chains double-buffered (2 banks) so S matmuls
        # never wait on a chain's CAST, s tiles 2x2 banks, ye/yo 2 banks
        psp = ctx.enter_context(tc.tile_pool(name="psp", bufs=2, space="PSUM"))
        ps = ctx.enter_context(tc.tile_pool(name="ps", bufs=2, space="PSUM"))
        psy = ctx.enter_context(tc.tile_pool(name="psy", bufs=1, space="PSUM"))

        # big packed input tiles: [128, cblk*W] with all 8 contraction
        # blocks side by side.  wqk is 4 separate per-ft tiles and xT
        # t-block 0 is 2 half tiles: the dep tracker merges adjacent DMA
        # write-ranges within one tile, which made early chain matmuls
        # wait on unrelated slices' transfers
        xT0h = [sb.tile([128, 2048], BF16, name=f"xT0h{i}") for i in range(2)]
        xTb = [None] + [sb.tile([128, 4096], BF16, name=f"xTb{tb}")
                        for tb in range(1, 4)]
        wqkf = [sb.tile([128, 1024], BF16, name=f"wqkf{ft}")
                for ft in range(4)]
        wvb = sb.tile([128, 2080], BF16, name="wvb")

        def xslice(tb, lo, hi):
            """moving/stationary slice of x^T t-block tb, cols [lo, hi)."""
            if tb == 0:
                return (xT0h[0][:, lo:hi] if hi <= 2048
                        else xT0h[1][:, lo - 2048:hi - 2048])
            return xTb[tb][:, lo:hi]
        qs = [[sb.tile([128, 512], BF16, name=f"q{p}_{tb}") for tb in range(4)]
              for p in range(2)]
        # kT per pair, unsplit: rows 0:64 even head, 64:128 odd head
        kts = [[sb.tile([128, 512], BF16, name=f"kt{p}_{tb}")
                for tb in range(4)] for p in range(2)]
        vs = [sb.tile([128, 260], BF16, name=f"v_{t}") for t in range(16)]
        ones2 = sb.tile([128, 4], F32, name="ones2")
        cmt = sb.tile([128, 128], BF16, name="cmt")

        # input DMAs.  All in-flight transfers FAIR-SHARE the per-core HBM
        # bandwidth (each dma_start lands on its own HW DMA queue), so the
        # critical-window byte count is what sets time-to-first-matmul:
        # only wqk (ft0/ft2 first), xT t-block 0, wv and the mask ride the
        # HW DGE rings; the late-needed xT t-blocks 1 and 3 go to the slow
        # gpsimd software DGE (~0.17 MB/us, barely touches HBM share) and
        # t-block 2 joins a HW ring behind the critical set.
        nc.scalar.dma_start(wqkf[0][:], wqk_d.ap()[:, 0:1024])
        nc.sync.dma_start(xT0h[0][:], xT_d.ap()[0, :, 0:2048])
        nc.scalar.dma_start(wqkf[2][:], wqk_d.ap()[:, 2048:3072])
        nc.sync.dma_start(xT0h[1][:], xT_d.ap()[0, :, 2048:4096])
        nc.scalar.dma_start(wqkf[1][:], wqk_d.ap()[:, 1024:2048])
        nc.scalar.dma_start(wqkf[3][:], wqk_d.ap()[:, 3072:4096])
        nc.sync.dma_start(wvb[:], wv_d.ap()[:, :])
        nc.sync.dma_start(cmt[:], cm_d.ap()[:, :])
        nc.gpsimd.dma_start(xTb[1][:], xT_d.ap()[1, :, :])
        nc.gpsimd.dma_start(xTb[3][:], xT_d.ap()[3, :, :])

        nc.gpsimd.memset(ones2[:], 1.0)

        # dummy activation so walrus's ACT_TABLE_LOAD (~2.7us incl drain)
        # runs during the DMA window instead of before the first real exp
        dumt = sb.tile([128, 4], F32, name="dumt")
        nc.scalar.activation(dumt[:], ones2[:], EXP, scale=0.125)

        # PE warmup during the DMA window: needs no input data, keeps the
        # HAM clock-gate at 2.4 GHz so the first chains don't run at the
        # cold 1.2 GHz rate.  The tiny result DMA keeps it live (unread
        # outputs get dead-code-eliminated).
        wupb = sb.tile([128, 512], BF16, name="wupb2")
        nc.vector.memset(wupb[:], 0.25)
        # 46 matmuls bridge the whole ~12us HBM-bound input wait (first
        # ~8 run at the cold 1.2 GHz rate, the rest at 2.4 GHz after the
        # HAM un-throttles): an idle gap >3.4us would re-throttle the PE
        # and the first real chains would run at half clock
        wups = psp.tile([128, 512], F32, name="wups", tag="pmm")
        for i in range(40):
            nc.tensor.matmul(wups[:], wupb[:, 0:128], wupb[:],
                             start=(i == 0), stop=(i == 39))
        wdbg = sb.tile([1, 12], F32, name="wdbg")
        nc.vector.tensor_copy(wdbg[:, 0:4], wups[0:1, 0:4])

        def qk_chain(p, ft_kind, tb):
            """One projection chain: q (ft_kind=0) or k (ft_kind=1) of pair p,
            t-block tb."""
            ft = p if ft_kind == 0 else 2 + p
            mm = psp.tile([128, 512], F32, name=f"pqk{p}_{ft}_{tb}", tag="pmm")
            for c in range(8):
                nc.tensor.matmul(mm[:],
                                 wqkf[ft][:, c * 128:(c + 1) * 128],
                                 xslice(tb, c * 512, (c + 1) * 512),
                                 start=(c == 0), stop=(c == 7))
            dst = qs if ft_kind == 0 else kts
            nc.vector.tensor_copy(dst[p][tb][:], mm[:])

        def v_chain(tt):
            """Combined v projection for one t-tile (all 4 heads, N=260)."""
            tb, sub = tt // 4, tt % 4
            mmv = psp.tile([128, 260], F32, name=f"pv{tt}", tag="pmm")
            for c in range(8):
                nc.tensor.matmul(mmv[:],
                                 xslice(tb, c * 512 + sub * 128,
                                        c * 512 + sub * 128 + 128),
                                 wvb[:, c * 260:(c + 1) * 260],
                                 start=(c == 0), stop=(c == 7))
            nc.vector.tensor_copy(vs[tt][:], mmv[:])
            nc.vector.tensor_copy(vs[tt][:, 64:260:65], ones2[:])

        ptiles = {}

        def attn_s_part(p, j, kk):
            """S matmuls + exp + causal mask for chunk (p, j, kk).

            The even/odd head S matmuls are K=64 row-tiled to PE rows
            0:63 / 64:127 and run CONCURRENTLY (tile_position derived
            from the operands' base partitions); outputs go to the two
            separate PSUM banks of the s tile.

            Diagonal k-tiles only have valid scores for q >= k, i.e. local
            q >= off = 128*(kk-4j); the matmuls/exp/PV all start at column
            off, and the mask multiply zeroes the partial 128-col block's
            lower triangle."""
            off = max(0, 128 * (kk - 4 * j))
            roff = off
            s = ps.tile([128, 1024], F32, name=f"s{p}_{j}_{kk}", tag="s")
            ktb, ksub = kk // 4, (kk % 4) * 128
            nc.tensor.matmul(s[:, roff:512],
                             kts[p][ktb][0:64, ksub:ksub + 128],
                             qs[p][j][0:64, roff:512],
                             start=True, stop=True)
            nc.tensor.matmul(s[:, 512 + roff:1024],
                             kts[p][ktb][64:128, ksub:ksub + 128],
                             qs[p][j][64:128, roff:512],
                             start=True, stop=True)
            pt = pp.tile([128, 1024], BF16, name=f"pt{p}_{j}_{kk}", tag="pt")
            sv = s[:].rearrange("p (b q) -> p b q", b=2)[:, :, roff:512]
            pv = pt[:].rearrange("p (b q) -> p b q", b=2)[:, :, roff:512]
            nc.scalar.activation(pv, sv, EXP, scale=0.125)
            if kk >= 4 * j:
                # causal mask both head halves in one GpSimd multiply over
                # just the partial 128-col diagonal block at [off, off+128)
                v3 = pt[:].rearrange("p (b q) -> p b q", b=2)[:, :,
                                                             off:off + 128]
                m3 = cmt[:].rearrange("p (b q) -> p b q", b=1).to_broadcast(
                    (128, 2, 128))
                nc.gpsimd.tensor_tensor(v3, v3, m3, op=MULT)
            ptiles[(p, j, kk)] = pt

        states = {}

        def finalize(p, j, state):
            """Ship the [65,512] accumulators (64 y rows + denominator row)
            out; the PSUM-freeing copies are all that's left on-device --
            the host divides y rows by the denominator row."""
            ye, yo = state["ye"], state["yo"]
            ysbe = yp.tile([65, 512], BF16, name=f"ysbe{p}_{j}", tag="ysbe")
            nc.vector.tensor_copy(ysbe[:], ye[:])
            nc.sync.dma_start(
                y_d.ap()[2 * p, :, j * 512:(j + 1) * 512], ysbe[:])
            ysbo = yp.tile([65, 512], BF16, name=f"ysbo{p}_{j}", tag="ysbo")
            nc.vector.tensor_copy(ysbo[:], yo[:])
            nc.sync.dma_start(
                y_d.ap()[2 * p + 1, :, j * 512:(j + 1) * 512], ysbo[:])

        def attn_pv_part(p, j, kk):
            """PV accumulation for chunk (p, j, kk); finishes the q-block
            with finalize on its last k-tile."""
            state = states.setdefault((p, j), {})
            nkt = 4 * (j + 1)
            if kk == 0:
                state["ye"] = psy.tile([65, 512], F32,
                                       name=f"ye{p}_{j}", tag="ye")
                state["yo"] = psy.tile([65, 512], F32,
                                       name=f"yo{p}_{j}", tag="yo")
            pt = ptiles.pop((p, j, kk))
            first, last = (kk == 0), (kk == nkt - 1)
            # skip columns left of off (all-zero P above the causal
            # diagonal); their y contribution is zero and PSUM keeps the
            # prior partials there
            roff = 0 if first else max(0, 128 * (kk - 4 * j))
            nc.tensor.matmul(state["ye"][:, roff:512],
                             vs[kk][:, 130 * p:130 * p + 65],
                             pt[:, roff:512],
                             start=first, stop=last)
            nc.tensor.matmul(state["yo"][:, roff:512],
                             vs[kk][:, 130 * p + 65:130 * p + 130],
                             pt[:, 512 + roff:1024],
                             start=first, stop=last)
            if last:
                finalize(p, j, state)

        # One flat software pipeline over BOTH pairs, block-interleaved:
        # (p0,b0),(p1,b0),(p0,b1),(p1,b1),...  PV lags S by 4 chunks so
        # ScalarE's exp queue stays full; projection chains are spread across
        # the PRECEDING sub-block's chunks as PE filler (work for (p1,t)
        # during (p0,t); work for (p0,t+1) and its v tiles during (p1,t)).
        seq = [(p, t, kk) for t in range(NQB) for p in range(2)
               for kk in range(4 * (t + 1))]
        work_during = {}
        for t in range(NQB):
            work_during[(0, t)] = [lambda t=t: qk_chain(1, 0, t),
                                   lambda t=t: qk_chain(1, 1, t)]
            if t < NQB - 1:
                work_during[(1, t)] = (
                    [lambda t=t: qk_chain(0, 0, t + 1),
                     lambda t=t: qk_chain(0, 1, t + 1)] +
                    [lambda tt=tt: v_chain(tt)
                     for tt in range(4 * (t + 1), 4 * (t + 1) + 4)])
        # pair-0 block-0's v tiles ride the (0,0) filler slots (after the
        # pair-1 projections, so wvb -- queued behind 2MB of critical DMA
        # -- has time to land); they must not sit upfront on the PE queue
        work_during[(0, 0)] = (work_during[(0, 0)] +
                               [lambda tt=tt: v_chain(tt)
                                for tt in range(4)])
        # upfront: pair-0 stage 0 projections only.  The non-critical
        # transfers (wqk ft1/ft3, xT t-block 2) are posted only after the
        # first chains' casts land: in-flight DMAs fair-share HBM, so an
        # early config would starve the critical startup data.  The
        # gating scalar copies write into the dbg output so dead-code
        # elimination keeps the delay chains.
        qk_chain(0, 0, 0)
        qk_chain(0, 1, 0)
        nc.scalar.copy(wdbg[:, 8:12], kts[0][0][0:1, 0:4])
        nc.scalar.dma_start(xTb[2][:], xT_d.ap()[2, :, :])
        nc.sync.dma_start(dbg_d.ap()[:, :], wdbg[:])

        # chunks are processed in PAIRS so same-shape matmuls sit adjacent
        # on the PE queue: [S(n) S(n+1)] [fillers] [PV(n-5) PV(n-4)].
        # Adjacent same-mode matmuls pipeline fill-under-drain (~N cycles);
        # isolated ones pay the exposed ~130ns drain.
        pend = []
        queue = []
        for (p, t, kk) in seq:
            if kk == 0:
                queue = list(work_during.get((p, t), ()))
            attn_s_part(p, t, kk)
            pend.append((p, t, kk))
            if kk % 2 == 1:
                # front-load filler: the CASTs must complete well before
                # the block boundary
                for _ in range(min(4, len(queue))):
                    queue.pop(0)()
                while len(pend) > 4:
                    pp_, pj, pkk = pend.pop(0)
                    attn_pv_part(pp_, pj, pkk)
        for pp_, pj, pkk in pend:
            attn_pv_part(pp_, pj, pkk)

    nc.compile()
    return nc


def _get_nc():
    global _NC
    if _NC is None:
        _NC = _build()
    return _NC


def _make_in_maps(x, W_attn):
    x = np.asarray(x, dtype=np.float32)
    W = np.asarray(W_attn, dtype=np.float32)
    wq, wk, wv = W[0:C], W[C:2 * C], W[2 * C:3 * C]
    cm = _causal_masks()
    in_maps = []
    for c in range(NCORES):
        b, g = c // 4, c % 4
        heads = [HPC * g + i for i in range(HPC)]
        # xT packed: [tb, p, cblk*512 + f]
        xT = np.ascontiguousarray(x[b].T)            # [C, T]
        xTr = xT.reshape(8, 128, 4, 512)             # [cblk, p, tb, f]
        xTbig = np.ascontiguousarray(
            xTr.transpose(2, 1, 0, 3).reshape(4, 128, 4096)).astype(NPBF16)
        qrows = np.concatenate([wq[D * h:D * h + D] for h in heads], axis=0)
        krows = np.concatenate([wk[D * h:D * h + D] for h in heads], axis=0)
        wqk_np = np.ascontiguousarray(
            np.concatenate([qrows, krows], 0).T)     # [C, 512]
        # ft-major packing: [p, ft*1024 + cblk*128 + f]
        wqkbig = np.ascontiguousarray(
            wqk_np.reshape(8, 128, 4, 128).transpose(1, 2, 0, 3)
            .reshape(128, 4096)).astype(NPBF16)
        wv_np = np.zeros((C, HPC * 65), np.float32)
        for i, h in enumerate(heads):
            wv_np[:, 65 * i:65 * i + D] = wv[D * h:D * h + D].T
        wvbig = np.ascontiguousarray(
            wv_np.reshape(8, 128, 260).transpose(1, 0, 2)
            .reshape(128, 2080)).astype(NPBF16)
        in_maps.append({"xT": xTbig, "wqk": wqkbig, "wv": wvbig, "cm": cm})
    return in_maps


def _causal_masks():
    r = np.arange(128)[:, None]
    return (np.arange(128)[None, :] >= r).astype(NPBF16)


def _execute(in_maps, trace=False):
    return run_bass_kernel_spmd(_get_nc(), in_maps,
                                core_ids=list(range(NCORES)), trace=trace)


def _assemble(results):
    y = np.empty((B, T, C), np.float32)
    for c in range(NCORES):
        b, g = c // 4, c % 4
        yc = results[c]["y"].astype(np.float32)       # [4, 65, 2048] bf16
        ynorm = yc[:, :64] / yc[:, 64:65]             # host-side softmax div
        y[b, :, 256 * g:256 * (g + 1)] = \
            ynorm.transpose(2, 0, 1).reshape(T, 256)
    return y


def kernel(x, W_attn):
    res = _execute(_make_in_maps(x, W_attn), trace=False)
    return _assemble(res.results)
